# revision 20
# baseline (speedup 1.0000x reference)
"""Trainium2 Bass kernel for nn_Memory_22548578304755 (scatter_memory).

Computes: mean_b [ -log_softmax(mask(inputs @ features.T / temp))[b, indices[b]] ]

Strategy (8 NeuronCores, SPMD):
  - Shard the feature bank row-wise: core c owns rows [c*12500, (c+1)*12500).
  - Host pre-transposes + casts each shard to fp16 [D, N/8] so matmul
    operands have the contraction dim (D) on SBUF partitions.
  - The intra-camera mask is folded into the matmul: 8 extra one-hot
    "camera" rows are appended to the contraction. The inputs side carries
    BIG * onehot(camids_batch), the features side carries onehot(camids).
    Matching camids add +BIG to the score; after the fixed shift
    exp(score - (BIG + K)) the non-matching entries underflow to 0 exactly.
  - Each core computes s_c[b] = sum_n exp(aug_score[b,n] - C_SHIFT) via
    PSUM -> ScalarE exp-with-accumulate; the host combines the 8 partial
    softmax denominators (cross-device logsumexp) and the on-device
    target score (masked row dot) into the final scalar.
"""

import sys

import numpy as np

sys.path.insert(0, "/opt/trn_rl_repo")

import concourse.bacc as bacc  # noqa: E402
import concourse.mybir as mybir  # noqa: E402
from concourse.tile import TileContext  # noqa: E402
from concourse.bass_utils import run_bass_kernel_spmd  # noqa: E402

B = 64
N = 100000
D = 2048
NCAMS = 8
TEMP = 0.07
NCORES = 8
N_SHARD = N // NCORES  # 12500

BIG = 1024.0  # mask offset added to same-camera scores (exact in fp16)
K_SHIFT = 100.0  # extra shift so exp never overflows
C_SHIFT = BIG + K_SHIFT

KC = D // 128  # 16 full contraction chunks
N_MM = 500  # matmul moving free-dim (one PSUM bank)
# DMA chunk schedule: small chunks first to fill the pipeline quickly,
# then large chunks for DMA efficiency. Must sum to N_SHARD, each a
# multiple of N_MM.
CHUNKS = (500,) + (1000,) * 11 + (500, 500)


def build_nc(n_shard: int, chunks=CHUNKS, n_mm: int = N_MM):
    """Build the single-core Bass program (identical across the 8 cores)."""
    assert sum(chunks) == n_shard and all(c % n_mm == 0 for c in chunks)
    max_chunk = max(chunks)
    total_mm = n_shard // n_mm

    dt = mybir.dt
    nc = bacc.Bacc()

    featT = nc.declare_dram_parameter("featT", [D, n_shard], dt.float16, False)
    featC = nc.declare_dram_parameter("featC", [NCAMS, n_shard], dt.float16, False)
    inpP = nc.declare_dram_parameter("inpP", [128, (KC + 1) * B], dt.float16, False)
    gath = nc.declare_dram_parameter("gath", [B, D + NCAMS], dt.float32, False)
    xnat = nc.declare_dram_parameter("xnat", [B, D + NCAMS], dt.float32, False)
    out = nc.declare_dram_parameter("out", [B, 2], dt.float32, True)

    with TileContext(nc) as tc:
        with (
            tc.tile_pool(name="feat", bufs=3) as featp,
            tc.tile_pool(name="small", bufs=1) as smallp,
            tc.tile_pool(name="scratch", bufs=3) as scrp,
            tc.tile_pool(name="psum", bufs=4, space="PSUM") as psump,
        ):
            inp_t = smallp.tile([128, (KC + 1) * B], dt.float16)
            nc.scalar.dma_start(inp_t[:], inpP[:, :])
            partials = smallp.tile([B, total_mm], dt.float32)
            out_sb = smallp.tile([B, 2], dt.float32)
            nbias = smallp.tile([B, 1], dt.float32)
            nc.vector.memset(nbias[:], -C_SHIFT)

            # Target-score row dot: tsel[b] = sum(gath[b] * xnat[b]).
            # Early, on the scalar HWDGE queue + DVE (both idle at the start).
            g_t = smallp.tile([B, D + NCAMS], dt.float32)
            x_t = smallp.tile([B, D + NCAMS], dt.float32)
            nc.scalar.dma_start(g_t[:], gath[:, :])
            nc.scalar.dma_start(x_t[:], xnat[:, :])
            nc.vector.tensor_mul(g_t[:], g_t[:], x_t[:])
            nc.vector.reduce_sum(
                out=out_sb[:, 1:2], in_=g_t[:], axis=mybir.AxisListType.X
            )

            mi = 0
            c0 = 0
            for ci, csz in enumerate(chunks):
                dma_eng = nc.sync if ci % 2 == 0 else nc.scalar
                ft = featp.tile([128, KC, max_chunk], dt.float16, tag="ft")
                src = featT[:, c0 : c0 + csz].rearrange("(kc p) n -> p kc n", p=128)
                dma_eng.dma_start(ft[:, :, :csz], src)
                camc = scrp.tile([NCAMS, max_chunk], dt.float16, tag="camc")
                dma_eng.dma_start(camc[:, :csz], featC[:, c0 : c0 + csz])
                for si in range(csz // n_mm):
                    ps = psump.tile([B, n_mm], dt.float32, tag="ps")
                    for k in range(KC):
                        nc.tensor.matmul(
                            ps[:],
                            inp_t[:, k * B : (k + 1) * B],
                            ft[:, k, si * n_mm : (si + 1) * n_mm],
                            start=(k == 0),
                            stop=False,
                        )
                    nc.tensor.matmul(
                        ps[:],
                        inp_t[0:NCAMS, KC * B : KC * B + B],
                        camc[:, si * n_mm : (si + 1) * n_mm],
                        start=False,
                        stop=True,
                    )
                    ex = scrp.tile([B, n_mm], dt.float32, tag="ex")
                    nc.scalar.activation(
                        ex[:],
                        ps[:],
                        mybir.ActivationFunctionType.Exp,
                        bias=nbias[:],
                        scale=1.0,
                        accum_out=partials[:, mi : mi + 1],
                    )
                    mi += 1
                c0 += csz

            nc.vector.reduce_sum(
                out=out_sb[:, 0:1], in_=partials[:], axis=mybir.AxisListType.X
            )
            nc.sync.dma_start(out[:, :], out_sb[:])
    nc.finalize()
    return nc


def _prep_host(inputs, features, indices, camids, camids_batch, n_shard):
    """Host-side shard prep. Returns per-core in_maps."""
    f16 = np.float16
    x = np.asarray(inputs, np.float32) / TEMP  # [B, D]
    cb = np.asarray(camids_batch).astype(np.int64)
    cn = np.asarray(camids).astype(np.int64)
    idx = np.asarray(indices).astype(np.int64)

    oh_b = (cb[:, None] == np.arange(NCAMS)[None, :]).astype(np.float32)  # [B, 8]

    # Packed lhsT: [128, 17*64]; block k<16 = x.T rows, block 16 = BIG*oh_b.T.
    inpP = np.zeros((128, (KC + 1) * B), np.float32)
    xt = x.T  # [D, B]
    for k in range(KC):
        inpP[:, k * B : (k + 1) * B] = xt[k * 128 : (k + 1) * 128, :]
    inpP[:NCAMS, KC * B : KC * B + B] = (BIG * oh_b).T
    inpP = np.ascontiguousarray(inpP.astype(f16))

    gathered = np.concatenate(
        [np.asarray(features, np.float32)[idx], oh_b], axis=1
    ).astype(np.float32)  # [B, D+8]
    xnat = np.concatenate([x, BIG * oh_b], axis=1).astype(np.float32)

    ncores = features.shape[0] // n_shard
    in_maps = []
    for c in range(ncores):
        sl = slice(c * n_shard, (c + 1) * n_shard)
        fT = np.ascontiguousarray(
            np.asarray(features[sl], np.float32).T.astype(f16)
        )  # [D, n_shard]
        fC = np.ascontiguousarray(
            (cn[sl][None, :] == np.arange(NCAMS)[:, None]).astype(f16)
        )  # [8, n_shard]
        in_maps.append(
            {"featT": fT, "featC": fC, "inpP": inpP, "gath": gathered, "xnat": xnat}
        )
    return in_maps


def _combine_host(results):
    """Cross-core logsumexp combine -> final scalar."""
    s = np.stack([r["out"][:, 0] for r in results]).astype(np.float64)  # [ncores, B]
    tsel = results[0]["out"][:, 1].astype(np.float64)  # [B] = score_target + BIG
    stot = s.sum(axis=0)
    lse = np.log(stot) + C_SHIFT  # = logsumexp of aug scores
    nll = lse - tsel
    return np.float32(nll.mean())


_NC_CACHE = {}


def _get_nc(n_shard, chunks, n_mm):
    key = (n_shard, tuple(chunks), n_mm)
    if key not in _NC_CACHE:
        _NC_CACHE[key] = build_nc(n_shard, chunks, n_mm)
    return _NC_CACHE[key]


def run_device(in_maps, n_shard, chunks=CHUNKS, n_mm=N_MM, **kwargs):
    nc = _get_nc(n_shard, chunks, n_mm)
    return run_bass_kernel_spmd(
        nc, in_maps, core_ids=list(range(len(in_maps))), **kwargs
    )


def kernel(inputs, features, indices, camids, camids_batch):
    in_maps = _prep_host(inputs, features, indices, camids, camids_batch, N_SHARD)
    res = run_device(in_maps, N_SHARD)
    return _combine_host(res.results)


# revision 27
# speedup vs baseline: 1.4849x; 1.4849x over previous
"""Trainium2 Bass kernel for nn_Memory_22548578304755 (scatter_memory).

Computes: mean_b [ -log_softmax(mask(inputs @ features.T / temp))[b, indices[b]] ]

Strategy (8 NeuronCores, SPMD):
  - Shard the feature bank row-wise: core c owns rows [c*12500, (c+1)*12500).
  - Host pre-transposes + casts each shard to fp16 [D, N/8] so matmul
    operands have the contraction dim (D) on SBUF partitions.
  - The intra-camera mask is folded into the matmul: 8 extra one-hot
    "camera" rows are appended to the contraction. The inputs side carries
    BIG * onehot(camids_batch), the features side carries onehot(camids).
    Matching camids add +BIG to the score; after the fixed shift
    exp(score - (BIG + K)) the non-matching entries underflow to 0 exactly.
  - Each core computes s_c[b] = sum_n exp(aug_score[b,n] - C_SHIFT) via
    PSUM -> ScalarE exp-with-accumulate; the host combines the 8 partial
    softmax denominators (cross-device logsumexp) and the on-device
    target score (masked row dot) into the final scalar.
"""

import sys

import numpy as np

sys.path.insert(0, "/opt/trn_rl_repo")

import ml_dtypes  # noqa: E402

import concourse.bacc as bacc  # noqa: E402
import concourse.mybir as mybir  # noqa: E402
from concourse.tile import TileContext  # noqa: E402
from concourse.bass_utils import run_bass_kernel_spmd  # noqa: E402

B = 64
N = 100000
D = 2048
NCAMS = 8
TEMP = 0.07
NCORES = 8
N_SHARD = N // NCORES  # 12500

BIG = 512.0  # mask offset added to same-camera scores (exact in fp16)
K_SHIFT = 100.0  # extra shift so exp never overflows
C_SHIFT = BIG + K_SHIFT
FEAT_SCALE = 64.0  # fp8 feature pre-scale (power of 2; BIG*FEAT_SCALE fp16-exact)

KC = D // 128  # 16 full contraction chunks
N_MM = 500  # matmul moving free-dim (one PSUM bank)
# DMA chunk schedule: small chunks first to fill the pipeline quickly,
# then large chunks for DMA efficiency. Must sum to N_SHARD, each a
# multiple of N_MM.
CHUNKS = (500,) + (1000,) * 11 + (500, 500)


def build_nc(n_shard: int, chunks=CHUNKS, n_mm: int = N_MM):
    """Build the single-core Bass program (identical across the 8 cores)."""
    assert sum(chunks) == n_shard and all(c % n_mm == 0 for c in chunks)
    max_chunk = max(chunks)
    total_mm = n_shard // n_mm

    dt = mybir.dt
    nc = bacc.Bacc()

    featT = nc.declare_dram_parameter("featT", [D, n_shard], dt.float8e3, False)
    featC = nc.declare_dram_parameter("featC", [NCAMS, n_shard], dt.float16, False)
    inpP = nc.declare_dram_parameter("inpP", [128, (KC + 1) * B], dt.float16, False)
    gath = nc.declare_dram_parameter("gath", [B, D + NCAMS], dt.float32, False)
    xnat = nc.declare_dram_parameter("xnat", [B, D + NCAMS], dt.float32, False)
    out = nc.declare_dram_parameter("out", [B, 2], dt.float32, True)

    with TileContext(nc) as tc:
        with (
            tc.tile_pool(name="feat", bufs=3) as featp,
            tc.tile_pool(name="small", bufs=1) as smallp,
            tc.tile_pool(name="scratch", bufs=3) as scrp,
            tc.tile_pool(name="psum", bufs=4, space="PSUM") as psump,
        ):
            inp_t = smallp.tile([128, (KC + 1) * B], dt.float16)
            nc.scalar.dma_start(inp_t[:], inpP[:, :])
            partials = smallp.tile([B, total_mm], dt.float32)
            out_sb = smallp.tile([B, 2], dt.float32)
            nbias = smallp.tile([B, 1], dt.float32)
            nc.vector.memset(nbias[:], -C_SHIFT)

            # Target-score row dot: tsel[b] = sum(gath[b] * xnat[b]).
            # Early, on the scalar HWDGE queue + DVE (both idle at the start).
            g_t = smallp.tile([B, D + NCAMS], dt.float32)
            x_t = smallp.tile([B, D + NCAMS], dt.float32)
            nc.scalar.dma_start(g_t[:], gath[:, :])
            nc.scalar.dma_start(x_t[:], xnat[:, :])
            nc.vector.tensor_mul(g_t[:], g_t[:], x_t[:])
            nc.vector.reduce_sum(
                out=out_sb[:, 1:2], in_=g_t[:], axis=mybir.AxisListType.X
            )

            mi = 0
            c0 = 0
            for ci, csz in enumerate(chunks):
                dma_eng = nc.sync if ci % 2 == 0 else nc.scalar
                ft = featp.tile([128, KC, max_chunk], dt.float8e3, tag="ft")
                src = featT[:, c0 : c0 + csz].rearrange("(kc p) n -> p kc n", p=128)
                dma_eng.dma_start(ft[:, :, :csz], src)
                camc = scrp.tile([NCAMS, max_chunk], dt.float16, tag="camc")
                dma_eng.dma_start(camc[:, :csz], featC[:, c0 : c0 + csz])
                for si in range(csz // n_mm):
                    ps = psump.tile([B, n_mm], dt.float32, tag="ps")
                    for k in range(KC):
                        nc.tensor.matmul(
                            ps[:],
                            inp_t[:, k * B : (k + 1) * B],
                            ft[:, k, si * n_mm : (si + 1) * n_mm],
                            start=(k == 0),
                            stop=False,
                        )
                    nc.tensor.matmul(
                        ps[:],
                        inp_t[0:NCAMS, KC * B : KC * B + B],
                        camc[:, si * n_mm : (si + 1) * n_mm],
                        start=False,
                        stop=True,
                    )
                    ex = scrp.tile([B, n_mm], dt.float32, tag="ex")
                    nc.scalar.activation(
                        ex[:],
                        ps[:],
                        mybir.ActivationFunctionType.Exp,
                        bias=nbias[:],
                        scale=1.0 / FEAT_SCALE,
                        accum_out=partials[:, mi : mi + 1],
                    )
                    mi += 1
                c0 += csz

            nc.vector.reduce_sum(
                out=out_sb[:, 0:1], in_=partials[:], axis=mybir.AxisListType.X
            )
            nc.sync.dma_start(out[:, :], out_sb[:])
    nc.finalize()
    return nc


def _prep_host(inputs, features, indices, camids, camids_batch, n_shard):
    """Host-side shard prep. Returns per-core in_maps."""
    f16 = np.float16
    x = np.asarray(inputs, np.float32) / TEMP  # [B, D]
    cb = np.asarray(camids_batch).astype(np.int64)
    cn = np.asarray(camids).astype(np.int64)
    idx = np.asarray(indices).astype(np.int64)

    oh_b = (cb[:, None] == np.arange(NCAMS)[None, :]).astype(np.float32)  # [B, 8]

    # Packed lhsT: [128, 17*64]; block k<16 = x.T rows, block 16 = BIG*oh_b.T.
    inpP = np.zeros((128, (KC + 1) * B), np.float32)
    xt = x.T  # [D, B]
    for k in range(KC):
        inpP[:, k * B : (k + 1) * B] = xt[k * 128 : (k + 1) * 128, :]
    inpP[:NCAMS, KC * B : KC * B + B] = (BIG * FEAT_SCALE * oh_b).T
    inpP = np.ascontiguousarray(inpP.astype(f16))

    gathered = np.concatenate(
        [np.asarray(features, np.float32)[idx], oh_b], axis=1
    ).astype(np.float32)  # [B, D+8]
    xnat = np.concatenate([x, BIG * oh_b], axis=1).astype(np.float32)

    ncores = features.shape[0] // n_shard
    in_maps = []
    for c in range(ncores):
        sl = slice(c * n_shard, (c + 1) * n_shard)
        fT = np.ascontiguousarray(
            (np.asarray(features[sl], np.float32).T * FEAT_SCALE).astype(
                ml_dtypes.float8_e3m4
            )
        )  # [D, n_shard]
        fC = np.ascontiguousarray(
            (cn[sl][None, :] == np.arange(NCAMS)[:, None]).astype(f16)
        )  # [8, n_shard]
        in_maps.append(
            {"featT": fT, "featC": fC, "inpP": inpP, "gath": gathered, "xnat": xnat}
        )
    return in_maps


def _combine_host(results):
    """Cross-core logsumexp combine -> final scalar."""
    s = np.stack([r["out"][:, 0] for r in results]).astype(np.float64)  # [ncores, B]
    tsel = results[0]["out"][:, 1].astype(np.float64)  # [B] = score_target + BIG
    stot = s.sum(axis=0)
    lse = np.log(stot) + C_SHIFT  # = logsumexp of aug scores
    nll = lse - tsel
    return np.float32(nll.mean())


_NC_CACHE = {}


def _get_nc(n_shard, chunks, n_mm):
    key = (n_shard, tuple(chunks), n_mm)
    if key not in _NC_CACHE:
        _NC_CACHE[key] = build_nc(n_shard, chunks, n_mm)
    return _NC_CACHE[key]


def run_device(in_maps, n_shard, chunks=CHUNKS, n_mm=N_MM, **kwargs):
    nc = _get_nc(n_shard, chunks, n_mm)
    return run_bass_kernel_spmd(
        nc, in_maps, core_ids=list(range(len(in_maps))), **kwargs
    )


def kernel(inputs, features, indices, camids, camids_batch):
    in_maps = _prep_host(inputs, features, indices, camids, camids_batch, N_SHARD)
    res = run_device(in_maps, N_SHARD)
    return _combine_host(res.results)


# revision 33
# speedup vs baseline: 1.8068x; 1.2168x over previous
"""Trainium2 Bass kernel for nn_Memory_22548578304755 (scatter_memory).

Computes: mean_b [ -log_softmax(mask(inputs @ features.T / temp))[b, indices[b]] ]

Strategy (8 NeuronCores, SPMD):
  - Shard the feature bank row-wise: core c owns rows [c*12500, (c+1)*12500),
    zero-padded to 13000 (padding columns produce exp(-C_SHIFT) = 0).
  - Host pre-transposes each shard to [D, 13000] and quantizes to
    fp8e3m4 scaled by 64 so matmul operands have the contraction dim (D)
    on SBUF partitions; the inputs operand stays fp16 (mixed-dtype matmul).
  - The intra-camera mask is folded into the matmul: 8 extra one-hot
    "camera" rows are appended to the contraction. The inputs side carries
    BIG*64 * onehot(camids_batch), the features side carries onehot(camids).
    Matching camids add +BIG to the (descaled) score; after the fixed shift
    exp(score - (BIG + K)) non-matching entries underflow to 0 exactly.
  - B=64 uses only half the PE array columns, so each chunk computes TWO
    500-column score groups concurrently via PE column tiling:
    tile_position (0,0) -> PSUM partitions 0..63, (0,64) -> 64..127.
  - Each core computes s_c[b] = sum_n exp(aug_score[b,n] - C_SHIFT) via
    PSUM -> ScalarE exp-with-accumulate (scale=1/64 descales); the host
    combines the 8 partial denominators (cross-device logsumexp) with the
    on-device fp32 target-score dot into the final scalar.
"""

import sys

import numpy as np

sys.path.insert(0, "/opt/trn_rl_repo")

import ml_dtypes  # noqa: E402

import concourse.bacc as bacc  # noqa: E402
import concourse.mybir as mybir  # noqa: E402
from concourse.tile import TileContext  # noqa: E402
from concourse.bass_utils import run_bass_kernel_spmd  # noqa: E402

B = 64
N = 100000
D = 2048
NCAMS = 8
TEMP = 0.07
NCORES = 8
N_SHARD_RAW = N // NCORES  # 12500
N_SHARD = 13000  # zero-padded so 500-col groups pair up for col-tiling

BIG = 512.0  # mask offset added to same-camera scores
K_SHIFT = 100.0  # extra shift so exp never overflows
C_SHIFT = BIG + K_SHIFT
FEAT_SCALE = 64.0  # fp8 feature pre-scale (power of 2; BIG*FEAT_SCALE fp16-exact)

KC = D // 128  # 16 full contraction chunks
N_MM = 500  # matmul moving free-dim (one PSUM bank)
CHUNKS = (1000,) * 13  # DMA chunk schedule; each chunk = one col-tiled pair


def build_nc(n_shard: int, chunks=CHUNKS, n_mm: int = N_MM):
    """Build the single-core Bass program (identical across the 8 cores)."""
    assert sum(chunks) == n_shard and all(c == 2 * n_mm for c in chunks)
    max_chunk = max(chunks)
    n_pairs = len(chunks)

    dt = mybir.dt
    nc = bacc.Bacc()

    featT = nc.declare_dram_parameter("featT", [D, n_shard], dt.float8e3, False)
    featC = nc.declare_dram_parameter("featC", [NCAMS, n_shard], dt.float16, False)
    inpP = nc.declare_dram_parameter("inpP", [128, (KC + 1) * B], dt.float16, False)
    gath = nc.declare_dram_parameter("gath", [B, D + NCAMS], dt.float32, False)
    xnat = nc.declare_dram_parameter("xnat", [B, D + NCAMS], dt.float32, False)
    out = nc.declare_dram_parameter("out", [128, 2], dt.float32, True)

    with TileContext(nc) as tc:
        with (
            tc.tile_pool(name="feat", bufs=3) as featp,
            tc.tile_pool(name="small", bufs=1) as smallp,
            tc.tile_pool(name="scratch", bufs=3) as scrp,
            tc.tile_pool(name="psum", bufs=4, space="PSUM") as psump,
        ):
            inp_t = smallp.tile([128, (KC + 1) * B], dt.float16)
            nc.scalar.dma_start(inp_t[:], inpP[:, :])
            partials = smallp.tile([128, n_pairs], dt.float32)
            out_sb = smallp.tile([128, 2], dt.float32)
            nbias = smallp.tile([128, 1], dt.float32)
            nc.vector.memset(nbias[:], -C_SHIFT)
            nc.vector.memset(out_sb[:], 0.0)

            # Target-score row dot: tsel[b] = sum(gath[b] * xnat[b]).
            g_t = smallp.tile([B, D + NCAMS], dt.float32)
            x_t = smallp.tile([B, D + NCAMS], dt.float32)
            nc.scalar.dma_start(g_t[:], gath[:, :])
            nc.scalar.dma_start(x_t[:], xnat[:, :])
            nc.vector.tensor_mul(g_t[:], g_t[:], x_t[:])
            nc.vector.reduce_sum(
                out=out_sb[0:B, 1:2], in_=g_t[:], axis=mybir.AxisListType.X
            )

            c0 = 0
            for ci, csz in enumerate(chunks):
                dma_eng = nc.sync if ci % 2 == 0 else nc.scalar
                ft = featp.tile([128, KC, max_chunk], dt.float8e3, tag="ft")
                src = featT[:, c0 : c0 + csz].rearrange("(kc p) n -> p kc n", p=128)
                dma_eng.dma_start(ft[:, :, :csz], src)
                camc = scrp.tile([NCAMS, max_chunk], dt.float16, tag="camc")
                dma_eng.dma_start(camc[:, :csz], featC[:, c0 : c0 + csz])

                ps = psump.tile([128, n_mm], dt.float32, tag="ps")
                for k in range(KC):
                    lhs = inp_t[:, k * B : (k + 1) * B]
                    nc.tensor.matmul(
                        ps[0:B, :],
                        lhs,
                        ft[:, k, 0:n_mm],
                        start=(k == 0),
                        stop=False,
                        tile_position=(0, 0),
                    )
                    nc.tensor.matmul(
                        ps[B : 2 * B, :],
                        lhs,
                        ft[:, k, n_mm : 2 * n_mm],
                        start=(k == 0),
                        stop=False,
                        tile_position=(0, B),
                    )
                lhs8 = inp_t[0:NCAMS, KC * B : KC * B + B]
                nc.tensor.matmul(
                    ps[0:B, :],
                    lhs8,
                    camc[:, 0:n_mm],
                    start=False,
                    stop=True,
                    tile_position=(0, 0),
                )
                nc.tensor.matmul(
                    ps[B : 2 * B, :],
                    lhs8,
                    camc[:, n_mm : 2 * n_mm],
                    start=False,
                    stop=True,
                    tile_position=(0, B),
                )
                ex = scrp.tile([128, n_mm], dt.float32, tag="ex")
                nc.scalar.activation(
                    ex[:],
                    ps[:],
                    mybir.ActivationFunctionType.Exp,
                    bias=nbias[:],
                    scale=1.0 / FEAT_SCALE,
                    accum_out=partials[:, ci : ci + 1],
                )
                c0 += csz

            nc.vector.reduce_sum(
                out=out_sb[:, 0:1], in_=partials[:], axis=mybir.AxisListType.X
            )
            nc.sync.dma_start(out[:, :], out_sb[:])
    nc.finalize()
    return nc


def _prep_host(inputs, features, indices, camids, camids_batch, n_shard_raw, n_shard):
    """Host-side shard prep. Returns per-core in_maps."""
    f16 = np.float16
    f8 = ml_dtypes.float8_e3m4
    x = np.asarray(inputs, np.float32) / TEMP  # [B, D]
    cb = np.asarray(camids_batch).astype(np.int64)
    cn = np.asarray(camids).astype(np.int64)
    idx = np.asarray(indices).astype(np.int64)

    oh_b = (cb[:, None] == np.arange(NCAMS)[None, :]).astype(np.float32)  # [B, 8]

    # Packed lhsT: [128, 17*64]; block k<16 = x.T rows, block 16 = aug rows.
    inpP = np.zeros((128, (KC + 1) * B), np.float32)
    xt = x.T  # [D, B]
    for k in range(KC):
        inpP[:, k * B : (k + 1) * B] = xt[k * 128 : (k + 1) * 128, :]
    inpP[:NCAMS, KC * B : KC * B + B] = (BIG * FEAT_SCALE * oh_b).T
    inpP = np.ascontiguousarray(inpP.astype(f16))

    gathered = np.concatenate(
        [np.asarray(features, np.float32)[idx], oh_b], axis=1
    ).astype(np.float32)  # [B, D+8]
    xnat = np.concatenate([x, BIG * oh_b], axis=1).astype(np.float32)

    ncores = features.shape[0] // n_shard_raw
    pad = n_shard - n_shard_raw
    in_maps = []
    for c in range(ncores):
        sl = slice(c * n_shard_raw, (c + 1) * n_shard_raw)
        fT = np.asarray(features[sl], np.float32).T * FEAT_SCALE  # [D, raw]
        if pad:
            fT = np.concatenate([fT, np.zeros((D, pad), np.float32)], axis=1)
        fT = np.ascontiguousarray(fT.astype(f8))
        fC = (cn[sl][None, :] == np.arange(NCAMS)[:, None]).astype(np.float32)
        if pad:
            fC = np.concatenate([fC, np.zeros((NCAMS, pad), np.float32)], axis=1)
        fC = np.ascontiguousarray(fC.astype(f16))
        in_maps.append(
            {"featT": fT, "featC": fC, "inpP": inpP, "gath": gathered, "xnat": xnat}
        )
    return in_maps


def _combine_host(results):
    """Cross-core logsumexp combine -> final scalar."""
    raw = np.stack([r["out"][:, 0] for r in results]).astype(np.float64)
    s = raw[:, :B] + raw[:, B:]  # [ncores, B] — add the two col-tile halves
    tsel = results[0]["out"][:B, 1].astype(np.float64)  # [B] = score_target + BIG
    stot = s.sum(axis=0)
    lse = np.log(stot) + C_SHIFT  # = logsumexp of aug scores
    nll = lse - tsel
    return np.float32(nll.mean())


_NC_CACHE = {}


def _get_nc(n_shard, chunks, n_mm):
    key = (n_shard, tuple(chunks), n_mm)
    if key not in _NC_CACHE:
        _NC_CACHE[key] = build_nc(n_shard, chunks, n_mm)
    return _NC_CACHE[key]


def run_device(in_maps, n_shard, chunks=CHUNKS, n_mm=N_MM, **kwargs):
    nc = _get_nc(n_shard, chunks, n_mm)
    return run_bass_kernel_spmd(
        nc, in_maps, core_ids=list(range(len(in_maps))), **kwargs
    )


def kernel(inputs, features, indices, camids, camids_batch):
    in_maps = _prep_host(
        inputs, features, indices, camids, camids_batch, N_SHARD_RAW, N_SHARD
    )
    res = run_device(in_maps, N_SHARD)
    return _combine_host(res.results)
